# revision 7
# baseline (speedup 1.0000x reference)
"""
Trainium2 (Bass/Tile) kernel for nn_ContextAwareAttentionBlock.

Problem shapes (hardcoded, from the problem spec):
    B=8, C=256, H=W=64  -> N = H*W = 4096 pixels per batch
    FD=32 (q/k feature dim), HID=128 (pooling MLP hidden dim)

Reference math:
    xf   = x.reshape(B, C, N)
    q,k,v = 1x1 convs of xf;  attn = softmax(q @ k);  sa_out = v @ attn^T
    h_sa = gamma * sa_out + x                      # gamma is a learned scalar
    hid  = tanh(fc_w @ h_sa + fc_b)                # [HID, N] per batch
    s    = softmax(ctx_w @ hid)  over N            # [N]    per batch
    out[b, c] = sum_n x[b, c, n] * s[b, n]         # [B, C]

Sharding: pure data-parallel over batch B across the 8 NeuronCores (one
batch element per core, full weights replicated) -- no cross-device
attention traffic; tiny per-core outputs gathered on host.

Fast path (gamma == 0): the module initializes gamma = zeros(1), so
h_sa == x *exactly* and the whole self-attention branch is multiplied by
zero. The device kernel then only needs the pooling MLP + softmax +
weighted sum. The softmax is computed in one streaming pass without a
global max: |score| <= sum|ctx_w| =: c0 because tanh output is in
[-1, 1], so we subtract the host-known constant c0 (any uniform shift
cancels exactly in the softmax ratio). Normalization by 1/sum(exp) and
the final sum over the 4 chunk-partials are done on host as part of the
unshard (O(C) work per core).

General path (gamma != 0, or a pathological ctx_w norm): exact NumPy
fallback implementing the full reference math.

Device dataflow per core (bf16 data / fp32 accumulation):
  - x [256, 4096] bf16 streams as 8 [128, 1024] tiles, all DMAs issued
    up-front: channel-half 0 on the sync HWDGE queue, half 1 on the
    vector HWDGE queue (parallel ~630ns issue slots), weights on the
    scalar queue -- no serial-issue stall on one sequencer.
  - hid = tanh(fc_w @ x + fc_b) per [128, 1024] chunk (4 matmuls into a
    2-bank PSUM tile, one wide tanh).
  - score matmul uses ctx_w REPLICATED into all 128 lhsT columns, so the
    [128, 1024] PSUM result has the score row broadcast across all
    partitions at the same PE cost as an M=1 matmul (cost = streamed
    columns). exp runs wide ([128, 1024], same wall time as [1, 1024])
    and its per-partition accumulator gives the softmax denominator; the
    broadcast stage of the old design disappears entirely.
  - weighted sum: one scalar_tensor_tensor per channel-half per chunk
    ([128, 1024] multiply with free-dim accumulate into part[:, chunk]).
  - PE warm-up burst of dummy bf16 matmuls fills the dead DMA window and
    ramps the PE p-state (0.65 -> 1.2 -> 2.4 GHz after 3us busy).
  - tail: two PE transposes pack part0/part1 into a [4, 256] row set,
    ACT copies PSUM->SBUF, single 4-descriptor store DMA (+ a 16-byte
    den DMA); host sums 4 partials and divides by the denominator.
  - Kernel tail is drain-only: Tile's stock tail (all-engine barrier +
    sem clears + barrier) costs ~5us and protects nothing here; NRT
    re-initializes semaphores per execution.
"""

import numpy as np

B, C, H, W = 8, 256, 64, 64
N = H * W          # 4096
FD = C // 8        # 32
HID = 128
U = 512            # pipeline unit width (pixels)
NU = N // U        # 8 units
NBIG = 4           # retained for test.py compatibility
N_CORES = 8
N_WARM = 4         # PE warm-up matmuls (512 cols each)

_FAST = {}  # memoized compiled program


def _build_fast_nc():
    """Build + compile the Bass/Tile program for the gamma==0 fast path.

    Per-core I/O (one batch element per core):
      x      [256, 4096] bf16  batch slice, channels-major
      wpack  [256, 256]  bf16  cols 0:128 = fc_w^T; rows 0:128 of cols
                               128:256 = ctx_w replicated into 128 cols
      bpack  [128, 2]    f32   col 0 = fc_b, col 1 = -c0 (all rows)
      out    [128, 16]   f32   per-unit weighted-sum partials:
                               col u = channels 0:128 of unit u,
                               col 8+u = channels 128:256 (host sums)
      den    [1, 8]      f32   per-unit exp sums (host sums)

    The kernel is a uniform pipeline over 8 units of 512 pixels each:
      hid(u)  = 2 matmuls into a [128,512] PSUM tile + tanh
      rest(u) = replicated-ctxw score matmul + exp (+den accum) +
                2 weighted-sum multiply-accumulates on DVE
    with one-unit skew (hid(u+1) emitted before rest(u)) so the PE never
    waits on the ACT round trip.
    """
    import concourse.bass as bass
    import concourse.bacc as bacc
    import concourse.tile as tile
    from concourse import mybir
    from concourse.vector_clock import ScopedClock

    f32 = mybir.dt.float32
    bf16 = mybir.dt.bfloat16
    AF = mybir.ActivationFunctionType
    ALU = mybir.AluOpType

    class _SlimTailTC(tile.TileContext):
        # Drain-only kernel tail (see module docstring).
        def _drain_and_barrier(self, tick_clock, wait_clock):
            drain_inst = self.nc.sync.drain()
            wait_clock.add_sem_waits(
                drain_inst.ins, ScopedClock({None: tick_clock.global_clock})
            )
            popped = self.nc._tile_sem_poison_stack.pop()
            assert popped is self._sem_poison

    nc = bacc.Bacc("TRN2", target_bir_lowering=False, debug=False, num_devices=1)

    x_d = nc.dram_tensor("x", [C, N], bf16, kind="ExternalInput")
    wpack_d = nc.dram_tensor("wpack", [C, 2 * HID], bf16, kind="ExternalInput")
    bpack_d = nc.dram_tensor("bpack", [HID, 2], f32, kind="ExternalInput")
    out_d = nc.dram_tensor("out", [128, 2 * NU], f32, kind="ExternalOutput")
    den_d = nc.dram_tensor("den", [1, NU], f32, kind="ExternalOutput")

    with _SlimTailTC(nc) as tc:
        with (
            tc.tile_pool(name="const", bufs=1) as cpool,
            tc.tile_pool(name="xc", bufs=12) as xpool,
            tc.tile_pool(name="hid", bufs=3) as hpool,
            tc.tile_pool(name="expv", bufs=3) as epool,
            tc.tile_pool(name="scr", bufs=3) as spool,
            tc.tile_pool(name="acc", bufs=1) as apool,
            tc.tile_pool(name="ps_h", bufs=3, space="PSUM") as ps_h,
            tc.tile_pool(name="ps_s", bufs=3, space="PSUM") as ps_s,
        ):
            # Warm-up constants on the (otherwise idle) gpsimd engine.
            ones1 = cpool.tile([1, 128], bf16)
            onesN = cpool.tile([1, U], bf16)
            nc.gpsimd.memset(ones1, 1.0)
            nc.gpsimd.memset(onesN, 1.0)

            # Weights on the scalar-engine HWDGE queue (its sequencer is
            # free until the first tanh; the ACT table load overlaps).
            wp0 = cpool.tile([128, 2 * HID], bf16)
            wp1 = cpool.tile([128, HID], bf16)
            bp = cpool.tile([HID, 2], f32)
            nc.scalar.dma_start(out=wp0, in_=wpack_d[0:128, :])
            nc.scalar.dma_start(out=wp1, in_=wpack_d[128:256, 0:HID])
            nc.scalar.dma_start(out=bp, in_=bpack_d[:, :])
            fcw0 = wp0[:, 0:HID]
            fcw1 = wp1[:, :]
            ctxw_rep = wp0[:, HID : 2 * HID]
            fcb = bp[:, 0:1]
            negc0 = bp[:, 1:2]

            # x DMA split across all three DMA-capable queues:
            #   sync   : units 0-3 as 8 x [128,512] (fine-grained so the
            #            pipeline fills as each 512-px slab lands)
            #   scalar : units 4-5 as 2 x [128,1024] (after the weights)
            #   gpsimd : units 6-7 as 2 x [128,1024] via SWDGE (slow
            #            issue + ~2.7us sem lag, fine for late units)
            xh0 = [None] * NU
            xh1 = [None] * NU
            for u in range(4):
                t0 = xpool.tile([128, U], bf16, tag=f"u{u}h0")
                t1 = xpool.tile([128, U], bf16, tag=f"u{u}h1")
                nc.sync.dma_start(out=t0, in_=x_d[0:128, bass.ts(u, U)])
                nc.sync.dma_start(out=t1, in_=x_d[128:256, bass.ts(u, U)])
                xh0[u], xh1[u] = t0, t1
            c2h0 = xpool.tile([128, 2 * U], bf16, tag="c2h0")
            c2h1 = xpool.tile([128, 2 * U], bf16, tag="c2h1")
            nc.scalar.dma_start(out=c2h0, in_=x_d[0:128, 2048:3072])
            nc.scalar.dma_start(out=c2h1, in_=x_d[128:256, 2048:3072])
            c3h0 = xpool.tile([128, 2 * U], bf16, tag="c3h0")
            c3h1 = xpool.tile([128, 2 * U], bf16, tag="c3h1")
            nc.gpsimd.dma_start(out=c3h0, in_=x_d[0:128, 3072:4096])
            nc.gpsimd.dma_start(out=c3h1, in_=x_d[128:256, 3072:4096])
            for u in (4, 5):
                s = slice((u - 4) * U, (u - 3) * U)
                xh0[u], xh1[u] = c2h0[:, s], c2h1[:, s]
            for u in (6, 7):
                s = slice((u - 6) * U, (u - 5) * U)
                xh0[u], xh1[u] = c3h0[:, s], c3h1[:, s]

            # PE warm-up: dummy bf16 matmuls fill the dead window while
            # the first x slabs stream in, and ramp the PE p-state.
            for _ in range(N_WARM):
                pw = ps_s.tile([128, U], f32, tag="psc")
                nc.tensor.matmul(pw, ones1, onesN, start=True, stop=True)

            den_parts = apool.tile([HID, NU], f32)
            part = apool.tile([128, 2 * NU], f32)

            def stage_hid(u):
                ph = ps_h.tile([HID, U], f32, tag="ph")
                nc.tensor.matmul(ph, fcw0, xh0[u], start=True, stop=False)
                nc.tensor.matmul(ph, fcw1, xh1[u], start=False, stop=True)
                hid = hpool.tile([HID, U], bf16, tag="hid")
                nc.scalar.activation(hid, ph, AF.Tanh, bias=fcb)
                return hid

            def stage_rest(u, hid):
                # score matmul with ctx_w replicated into all 128 lhsT
                # columns: the PSUM tile holds the score row broadcast
                # across all 128 partitions (same PE cost as M=1).
                psc = ps_s.tile([128, U], f32, tag="psc")
                nc.tensor.matmul(psc, ctxw_rep, hid, start=True, stop=True)
                ex = epool.tile([128, U], bf16, tag="ex")
                nc.scalar.activation(
                    ex, psc, AF.Exp,
                    bias=negc0,
                    accum_out=den_parts[:, u : u + 1],
                )
                # part[c, u] = sum_n x[c, n] * e[n] (mul + accum on DVE)
                s0 = spool.tile([128, U], bf16, tag="s0")
                s1 = spool.tile([128, U], bf16, tag="s1")
                nc.vector.scalar_tensor_tensor(
                    out=s0, in0=xh0[u], scalar=1.0, in1=ex,
                    op0=ALU.mult, op1=ALU.mult,
                    accum_out=part[:, u : u + 1],
                )
                nc.vector.scalar_tensor_tensor(
                    out=s1, in0=xh1[u], scalar=1.0, in1=ex,
                    op0=ALU.mult, op1=ALU.mult,
                    accum_out=part[:, NU + u : NU + u + 1],
                )
                if u == 2:
                    # perf probe: identical stt with no accum_out, in a
                    # DVE slack window — tests whether the 2x/4x DVE
                    # perf modes engage without the accumulator.
                    sp = spool.tile([128, U], bf16, tag="s0")
                    nc.vector.scalar_tensor_tensor(
                        out=sp, in0=xh0[u], scalar=1.0, in1=ex,
                        op0=ALU.mult, op1=ALU.mult,
                    )

            prev = stage_hid(0)
            for u in range(1, NU):
                cur = stage_hid(u)
                stage_rest(u - 1, prev)
                prev = cur
            stage_rest(NU - 1, prev)

            # Tail: ship the raw partials; host does the O(C) reduction.
            nc.sync.dma_start(out=den_d[:, :], in_=den_parts[0:1, :])
            nc.sync.dma_start(out=out_d[:, :], in_=part)

    nc.compile()
    return nc


def _get_fast_nc():
    if "nc" not in _FAST:
        _FAST["nc"] = _build_fast_nc()
    return _FAST["nc"]


def _make_in_maps(xf, fc_w, fc_b, ctx_w):
    import ml_dtypes

    bf16 = ml_dtypes.bfloat16
    wpack = np.zeros((C, 2 * HID), dtype=bf16)
    wpack[:, 0:HID] = fc_w.T.astype(bf16)
    wpack[0:HID, HID : 2 * HID] = np.broadcast_to(
        ctx_w.reshape(HID, 1).astype(bf16), (HID, HID)
    )
    bpack = np.zeros((HID, 2), dtype=np.float32)
    bpack[:, 0] = fc_b
    bpack[:, 1] = -float(np.abs(ctx_w).sum())
    x_bf = np.ascontiguousarray(xf).astype(bf16)
    return [
        {"x": x_bf[b], "wpack": wpack, "bpack": bpack}
        for b in range(x_bf.shape[0])
    ]


def _fast_path(xf, fc_w, fc_b, ctx_w, trace=False):
    """xf: [B, C, N] f32. Returns [B, C] f32 (and BassKernelResults if trace)."""
    from concourse.bass_utils import run_bass_kernel_spmd

    nc = _get_fast_nc()
    in_maps = _make_in_maps(xf, fc_w, fc_b, ctx_w)
    res = run_bass_kernel_spmd(nc, in_maps, list(range(N_CORES)), trace=trace)
    out = np.empty((B, C), dtype=np.float32)
    for b in range(B):
        parts = np.asarray(res.results[b]["out"], dtype=np.float32)  # [128, 16]
        den = np.asarray(res.results[b]["den"], dtype=np.float32).sum()
        out[b, 0:128] = parts[:, 0:NU].sum(axis=1) / den
        out[b, 128:256] = parts[:, NU : 2 * NU].sum(axis=1) / den
    if trace:
        return out, res
    return out


def _general_path(x, wq, bq, wk, bk, wv, bv, gamma, fc_w, fc_b, ctx_w):
    """Exact NumPy implementation of the full reference (any gamma)."""
    x = np.asarray(x, np.float32)
    b, c, h, w = x.shape
    n = h * w
    xf = x.reshape(b, c, n)
    out = np.empty((b, c), dtype=np.float32)
    for i in range(b):
        xi = xf[i]  # [C, N]
        q = (wq @ xi).T + bq[None, :]            # [N, FD]
        k = (wk @ xi) + bk[:, None]              # [FD, N]
        logits = q @ k                           # [N, N]
        logits -= logits.max(axis=1, keepdims=True)
        e = np.exp(logits, dtype=np.float32)
        attn = e / e.sum(axis=1, keepdims=True)
        v = (wv @ xi) + bv[:, None]              # [C, N]
        sa = v @ attn.T                          # [C, N]
        h_sa = gamma.reshape(-1)[0] * sa + xi    # [C, N]
        hid = np.tanh(fc_w @ h_sa + fc_b[:, None])   # [HID, N]
        s = (ctx_w @ hid).reshape(n)             # [N]
        s = s - s.max()
        es = np.exp(s, dtype=np.float32)
        p = es / es.sum()
        out[i] = xi @ p
    return out


def kernel(**inputs):
    x = np.asarray(inputs["style_features"], np.float32)
    gamma = np.asarray(inputs["gamma"], np.float32)
    fc_w = np.asarray(inputs["fc_w"], np.float32)
    fc_b = np.asarray(inputs["fc_b"], np.float32)
    ctx_w = np.asarray(inputs["ctx_w"], np.float32)

    assert x.shape == (B, C, H, W), f"unexpected shape {x.shape}"
    c0 = float(np.abs(ctx_w).sum())

    if np.all(gamma == 0.0) and c0 <= 40.0 and np.isfinite(c0):
        # gamma == 0  =>  h_sa == x exactly; attention branch contributes 0.
        xf = x.reshape(B, C, N)
        return _fast_path(xf, fc_w, fc_b, ctx_w)

    return _general_path(
        x,
        np.asarray(inputs["wq"], np.float32),
        np.asarray(inputs["bq"], np.float32),
        np.asarray(inputs["wk"], np.float32),
        np.asarray(inputs["bk"], np.float32),
        np.asarray(inputs["wv"], np.float32),
        np.asarray(inputs["bv"], np.float32),
        gamma,
        fc_w,
        fc_b,
        ctx_w,
    )


# revision 19
# speedup vs baseline: 1.0307x; 1.0307x over previous
"""
Trainium2 (Bass/Tile) kernel for nn_ContextAwareAttentionBlock.

Problem shapes (hardcoded, from the problem spec):
    B=8, C=256, H=W=64  -> N = H*W = 4096 pixels per batch
    FD=32 (q/k feature dim), HID=128 (pooling MLP hidden dim)

Reference math:
    xf   = x.reshape(B, C, N)
    q,k,v = 1x1 convs of xf;  attn = softmax(q @ k);  sa_out = v @ attn^T
    h_sa = gamma * sa_out + x                      # gamma is a learned scalar
    hid  = tanh(fc_w @ h_sa + fc_b)                # [HID, N] per batch
    s    = softmax(ctx_w @ hid)  over N            # [N]    per batch
    out[b, c] = sum_n x[b, c, n] * s[b, n]         # [B, C]

Sharding: pure data-parallel over batch B across the 8 NeuronCores (one
batch element per core, full weights replicated) -- no cross-device
attention traffic; tiny per-core outputs gathered on host.

Fast path (gamma == 0): the module initializes gamma = zeros(1), so
h_sa == x *exactly* and the whole self-attention branch is multiplied by
zero. The device kernel then only needs the pooling MLP + softmax +
weighted sum. The softmax is computed in one streaming pass without a
global max: |score| <= sum|ctx_w| =: c0 because tanh output is in
[-1, 1], so we subtract the host-known constant c0 (any uniform shift
cancels exactly in the softmax ratio). Normalization by 1/sum(exp) and
the final sum over the 4 chunk-partials are done on host as part of the
unshard (O(C) work per core).

General path (gamma != 0, or a pathological ctx_w norm): exact NumPy
fallback implementing the full reference math.

Device dataflow per core (bf16 data / fp32 accumulation):
  - x [256, 4096] bf16 streams as 8 [128, 1024] tiles, all DMAs issued
    up-front: channel-half 0 on the sync HWDGE queue, half 1 on the
    vector HWDGE queue (parallel ~630ns issue slots), weights on the
    scalar queue -- no serial-issue stall on one sequencer.
  - hid = tanh(fc_w @ x + fc_b) per [128, 1024] chunk (4 matmuls into a
    2-bank PSUM tile, one wide tanh).
  - score matmul uses ctx_w REPLICATED into all 128 lhsT columns, so the
    [128, 1024] PSUM result has the score row broadcast across all
    partitions at the same PE cost as an M=1 matmul (cost = streamed
    columns). exp runs wide ([128, 1024], same wall time as [1, 1024])
    and its per-partition accumulator gives the softmax denominator; the
    broadcast stage of the old design disappears entirely.
  - weighted sum: one scalar_tensor_tensor per channel-half per chunk
    ([128, 1024] multiply with free-dim accumulate into part[:, chunk]).
  - PE warm-up burst of dummy bf16 matmuls fills the dead DMA window and
    ramps the PE p-state (0.65 -> 1.2 -> 2.4 GHz after 3us busy).
  - tail: two PE transposes pack part0/part1 into a [4, 256] row set,
    ACT copies PSUM->SBUF, single 4-descriptor store DMA (+ a 16-byte
    den DMA); host sums 4 partials and divides by the denominator.
  - Kernel tail is drain-only: Tile's stock tail (all-engine barrier +
    sem clears + barrier) costs ~5us and protects nothing here; NRT
    re-initializes semaphores per execution.
"""

import numpy as np

B, C, H, W = 8, 256, 64, 64
N = H * W          # 4096
FD = C // 8        # 32
HID = 128
# Pipeline unit widths (pixels). 256-px units at both ends shorten the
# fill latency (first tanh starts sooner) and the drain chain (last
# unit's tanh->score->exp->multiply chain is half as long); 512-px units
# in the middle keep per-op overhead low.
UNITS = (256, 256, 512, 512, 512, 512, 512, 512, 256, 256)
NU = len(UNITS)
NBIG = 4           # retained for test.py compatibility
N_CORES = 8
N_WARM = 4         # PE warm-up matmuls (512 cols each)

_FAST = {}  # memoized compiled program


def _build_fast_nc():
    """Build + compile the Bass/Tile program for the gamma==0 fast path.

    Per-core I/O (one batch element per core):
      x      [256, 4096] bf16  batch slice, channels-major
      wpack  [256, 256]  bf16  cols 0:128 = fc_w^T; rows 0:128 of cols
                               128:256 = ctx_w replicated into 128 cols
      bpack  [128, 2]    f32   col 0 = fc_b, col 1 = -c0 (all rows)
      out    [128, 20]   f32   per-unit weighted-sum partials:
                               col 2u = channels 0:128 of unit u,
                               col 2u+1 = channels 128:256 (host sums)
      den    [1, 10]     f32   per-unit exp sums (host sums)

    The kernel is a pipeline over pixel units (UNITS widths):
      hid(u)  = 2 matmuls into a [128,w] PSUM tile + tanh
      rest(u) = replicated-ctxw score matmul + exp + gpsimd den-reduce +
                2 weighted-sum multiply-accumulates on DVE
    with one-unit skew (hid(u+1) emitted before rest(u)) so the PE never
    waits on the ACT round trip. All x DMAs go on the sync queue in
    need-order (c3 on scalar) -- spreading them over queues lets
    late-needed data jump the shared DMA wire ahead of early-needed
    data (measured 2.4us PE stall).
    """
    import concourse.bass as bass
    import concourse.bacc as bacc
    import concourse.tile as tile
    from concourse import mybir
    from concourse.vector_clock import ScopedClock

    f32 = mybir.dt.float32
    bf16 = mybir.dt.bfloat16
    AF = mybir.ActivationFunctionType
    ALU = mybir.AluOpType

    class _SlimTailTC(tile.TileContext):
        # Drain-only kernel tail (see module docstring).
        def _drain_and_barrier(self, tick_clock, wait_clock):
            drain_inst = self.nc.sync.drain()
            wait_clock.add_sem_waits(
                drain_inst.ins, ScopedClock({None: tick_clock.global_clock})
            )
            popped = self.nc._tile_sem_poison_stack.pop()
            assert popped is self._sem_poison

    nc = bacc.Bacc("TRN2", target_bir_lowering=False, debug=False, num_devices=1)

    x_d = nc.dram_tensor("x", [C, N], bf16, kind="ExternalInput")
    wpack_d = nc.dram_tensor("wpack", [C, 2 * HID], bf16, kind="ExternalInput")
    bpack_d = nc.dram_tensor("bpack", [HID, 2], f32, kind="ExternalInput")
    out_d = nc.dram_tensor("out", [128, 2 * NU], f32, kind="ExternalOutput")
    den_d = nc.dram_tensor("den", [1, N], bf16, kind="ExternalOutput")

    # pixel offset of each unit
    offs = [0]
    for w in UNITS:
        offs.append(offs[-1] + w)
    assert offs[-1] == N

    with _SlimTailTC(nc) as tc:
        with (
            tc.tile_pool(name="const", bufs=1) as cpool,
            tc.tile_pool(name="xc", bufs=1) as xpool,
            tc.tile_pool(name="hid", bufs=3) as hpool,
            tc.tile_pool(name="scr", bufs=3) as spool,
            tc.tile_pool(name="acc", bufs=1) as apool,
            tc.tile_pool(name="ps_h", bufs=3, space="PSUM") as ps_h,
            tc.tile_pool(name="ps_s", bufs=3, space="PSUM") as ps_s,
        ):
            # Warm-up constants on the (otherwise idle) gpsimd engine.
            ones1 = cpool.tile([1, 128], bf16)
            onesN = cpool.tile([1, 512], bf16)
            nc.gpsimd.memset(ones1, 1.0)
            nc.gpsimd.memset(onesN, 1.0)

            # Weights + the last x chunk on the scalar-engine HWDGE queue
            # (its sequencer is free until the first tanh; the ACT table
            # load overlaps the issue slots).
            wp0 = cpool.tile([128, 2 * HID], bf16)
            wp1 = cpool.tile([128, HID], bf16)
            bp = cpool.tile([HID, 2], f32)
            nc.scalar.dma_start(out=wp0, in_=wpack_d[0:128, :])
            nc.scalar.dma_start(out=wp1, in_=wpack_d[128:256, 0:HID])
            nc.scalar.dma_start(out=bp, in_=bpack_d[:, :])
            fcw0 = wp0[:, 0:HID]
            fcw1 = wp1[:, :]
            ctxw_rep = wp0[:, HID : 2 * HID]
            fcb = bp[:, 0:1]
            negc0 = bp[:, 1:2]

            # x tiles. Sync queue carries everything up to px 3072 in
            # need-order; the last 1024 px ride the scalar queue (its
            # descriptors may jump the wire, which is harmless for
            # late-needed data).
            def xpair(name, lo, hi, engine):
                t0 = xpool.tile([128, hi - lo], bf16, tag=f"{name}h0")
                t1 = xpool.tile([128, hi - lo], bf16, tag=f"{name}h1")
                engine.dma_start(out=t0, in_=x_d[0:128, lo:hi])
                engine.dma_start(out=t1, in_=x_d[128:256, lo:hi])
                return t0, t1

            t0p = xpair("t0", 0, 256, nc.sync)
            t1p = xpair("t1", 256, 512, nc.sync)
            ap = xpair("a", 512, 1536, nc.sync)
            bpx = xpair("b", 1536, 2560, nc.sync)
            cp = xpair("c", 2560, 3072, nc.sync)
            dp = xpair("d", 3072, 4096, nc.scalar)

            # unit -> (xh0 view, xh1 view)
            tiles = {0: (t0p, 0), 1: (t1p, 256), 2: (ap, 512), 3: (ap, 512),
                     4: (bpx, 1536), 5: (bpx, 1536), 6: (cp, 2560),
                     7: (dp, 3072), 8: (dp, 3072), 9: (dp, 3072)}
            xh0 = [None] * NU
            xh1 = [None] * NU
            for u in range(NU):
                (th0, th1), base = tiles[u]
                s = slice(offs[u] - base, offs[u + 1] - base)
                xh0[u], xh1[u] = th0[:, s], th1[:, s]

            # PE warm-up: dummy bf16 matmuls fill the dead window while
            # the first x slabs stream in, and ramp the PE p-state.
            for _ in range(N_WARM):
                pw = ps_s.tile([128, 512], f32, tag="psc")
                nc.tensor.matmul(pw, ones1, onesN, start=True, stop=True)

            # ACT warm-up: absorb the first-op ramp (~110ns/op) off the
            # critical path, while the x DMAs are still in flight.
            wsrc = cpool.tile([128, 8], bf16)
            nc.gpsimd.memset(wsrc, 0.25)
            wdst = cpool.tile([128, 8], bf16)
            nc.scalar.activation(wdst, wsrc, AF.Tanh)
            nc.scalar.activation(wdst, wsrc, AF.Exp)

            # exp values land in a persistent arena; all 128 partitions
            # hold the same row, so row 0 ships out once at the end and
            # the host sums it for the softmax denominator (zero device
            # ops for den).
            ex_arena = apool.tile([128, N], bf16)
            part = apool.tile([128, 2 * NU], f32)

            def stage_hid(u):
                w = UNITS[u]
                ph = ps_h.tile([HID, w], f32, tag="ph")
                nc.tensor.matmul(ph, fcw0, xh0[u], start=True, stop=False)
                nc.tensor.matmul(ph, fcw1, xh1[u], start=False, stop=True)
                hid = hpool.tile([HID, w], bf16, tag="hid")
                nc.scalar.activation(hid, ph, AF.Tanh, bias=fcb)
                return hid

            def stage_rest(u, hid):
                w = UNITS[u]
                # score matmul with ctx_w replicated into all 128 lhsT
                # columns: the PSUM tile holds the score row broadcast
                # across all 128 partitions (same PE cost as M=1).
                psc = ps_s.tile([128, w], f32, tag="psc")
                nc.tensor.matmul(psc, ctxw_rep, hid, start=True, stop=True)
                ex = ex_arena[:, offs[u] : offs[u + 1]]
                nc.scalar.activation(ex, psc, AF.Exp, bias=negc0)
                # part[c, 2u+h] = sum_n x[c, n] * e[n] (mul + accum, DVE)
                s0 = spool.tile([128, w], bf16, tag="s0")
                s1 = spool.tile([128, w], bf16, tag="s1")
                nc.vector.scalar_tensor_tensor(
                    out=s0, in0=xh0[u], scalar=1.0, in1=ex,
                    op0=ALU.mult, op1=ALU.mult,
                    accum_out=part[:, 2 * u : 2 * u + 1],
                )
                nc.vector.scalar_tensor_tensor(
                    out=s1, in0=xh1[u], scalar=1.0, in1=ex,
                    op0=ALU.mult, op1=ALU.mult,
                    accum_out=part[:, 2 * u + 1 : 2 * u + 2],
                )

            prev = stage_hid(0)
            for u in range(1, NU):
                cur = stage_hid(u)
                stage_rest(u - 1, prev)
                prev = cur
            stage_rest(NU - 1, prev)

            # Tail: ship the raw partials; host does the O(C) reduction.
            # The bulk of part goes out as soon as unit NU-2 is done;
            # only the last unit's two columns ride the final DMA.
            nc.sync.dma_start(out=den_d[:, :], in_=ex_arena[0:1, :])
            nc.sync.dma_start(
                out=out_d[:, 0 : 2 * NU - 2], in_=part[:, 0 : 2 * NU - 2]
            )
            nc.sync.dma_start(
                out=out_d[:, 2 * NU - 2 : 2 * NU],
                in_=part[:, 2 * NU - 2 : 2 * NU],
            )

    nc.compile()
    return nc


def _get_fast_nc():
    if "nc" not in _FAST:
        _FAST["nc"] = _build_fast_nc()
    return _FAST["nc"]


def _make_in_maps(xf, fc_w, fc_b, ctx_w):
    import ml_dtypes

    bf16 = ml_dtypes.bfloat16
    wpack = np.zeros((C, 2 * HID), dtype=bf16)
    wpack[:, 0:HID] = fc_w.T.astype(bf16)
    wpack[0:HID, HID : 2 * HID] = np.broadcast_to(
        ctx_w.reshape(HID, 1).astype(bf16), (HID, HID)
    )
    bpack = np.zeros((HID, 2), dtype=np.float32)
    bpack[:, 0] = fc_b
    bpack[:, 1] = -float(np.abs(ctx_w).sum())
    x_bf = np.ascontiguousarray(xf).astype(bf16)
    return [
        {"x": x_bf[b], "wpack": wpack, "bpack": bpack}
        for b in range(x_bf.shape[0])
    ]


def _fast_path(xf, fc_w, fc_b, ctx_w, trace=False):
    """xf: [B, C, N] f32. Returns [B, C] f32 (and BassKernelResults if trace)."""
    from concourse.bass_utils import run_bass_kernel_spmd

    nc = _get_fast_nc()
    in_maps = _make_in_maps(xf, fc_w, fc_b, ctx_w)
    res = run_bass_kernel_spmd(nc, in_maps, list(range(N_CORES)), trace=trace)
    out = np.empty((B, C), dtype=np.float32)
    for b in range(B):
        parts = np.asarray(res.results[b]["out"], dtype=np.float32)  # [128, 2*NU]
        den = np.asarray(res.results[b]["den"]).astype(np.float64).sum()
        out[b, 0:128] = parts[:, 0::2].sum(axis=1) / den
        out[b, 128:256] = parts[:, 1::2].sum(axis=1) / den
    if trace:
        return out, res
    return out


def _general_path(x, wq, bq, wk, bk, wv, bv, gamma, fc_w, fc_b, ctx_w):
    """Exact NumPy implementation of the full reference (any gamma)."""
    x = np.asarray(x, np.float32)
    b, c, h, w = x.shape
    n = h * w
    xf = x.reshape(b, c, n)
    out = np.empty((b, c), dtype=np.float32)
    for i in range(b):
        xi = xf[i]  # [C, N]
        q = (wq @ xi).T + bq[None, :]            # [N, FD]
        k = (wk @ xi) + bk[:, None]              # [FD, N]
        logits = q @ k                           # [N, N]
        logits -= logits.max(axis=1, keepdims=True)
        e = np.exp(logits, dtype=np.float32)
        attn = e / e.sum(axis=1, keepdims=True)
        v = (wv @ xi) + bv[:, None]              # [C, N]
        sa = v @ attn.T                          # [C, N]
        h_sa = gamma.reshape(-1)[0] * sa + xi    # [C, N]
        hid = np.tanh(fc_w @ h_sa + fc_b[:, None])   # [HID, N]
        s = (ctx_w @ hid).reshape(n)             # [N]
        s = s - s.max()
        es = np.exp(s, dtype=np.float32)
        p = es / es.sum()
        out[i] = xi @ p
    return out


def kernel(**inputs):
    x = np.asarray(inputs["style_features"], np.float32)
    gamma = np.asarray(inputs["gamma"], np.float32)
    fc_w = np.asarray(inputs["fc_w"], np.float32)
    fc_b = np.asarray(inputs["fc_b"], np.float32)
    ctx_w = np.asarray(inputs["ctx_w"], np.float32)

    assert x.shape == (B, C, H, W), f"unexpected shape {x.shape}"
    c0 = float(np.abs(ctx_w).sum())

    if np.all(gamma == 0.0) and c0 <= 40.0 and np.isfinite(c0):
        # gamma == 0  =>  h_sa == x exactly; attention branch contributes 0.
        xf = x.reshape(B, C, N)
        return _fast_path(xf, fc_w, fc_b, ctx_w)

    return _general_path(
        x,
        np.asarray(inputs["wq"], np.float32),
        np.asarray(inputs["bq"], np.float32),
        np.asarray(inputs["wk"], np.float32),
        np.asarray(inputs["bk"], np.float32),
        np.asarray(inputs["wv"], np.float32),
        np.asarray(inputs["bv"], np.float32),
        gamma,
        fc_w,
        fc_b,
        ctx_w,
    )


# revision 20
# speedup vs baseline: 1.1054x; 1.0724x over previous
"""
Trainium2 (Bass/Tile) kernel for nn_ContextAwareAttentionBlock.

Problem shapes (hardcoded, from the problem spec):
    B=8, C=256, H=W=64  -> N = H*W = 4096 pixels per batch
    FD=32 (q/k feature dim), HID=128 (pooling MLP hidden dim)

Reference math:
    xf   = x.reshape(B, C, N)
    q,k,v = 1x1 convs of xf;  attn = softmax(q @ k);  sa_out = v @ attn^T
    h_sa = gamma * sa_out + x                      # gamma is a learned scalar
    hid  = tanh(fc_w @ h_sa + fc_b)                # [HID, N] per batch
    s    = softmax(ctx_w @ hid)  over N            # [N]    per batch
    out[b, c] = sum_n x[b, c, n] * s[b, n]         # [B, C]

Sharding: pure data-parallel over batch B across the 8 NeuronCores (one
batch element per core, full weights replicated) -- no cross-device
attention traffic; tiny per-core outputs gathered on host.

Fast path (gamma == 0): the module initializes gamma = zeros(1), so
h_sa == x *exactly* and the whole self-attention branch is multiplied by
zero. The device kernel then only needs the pooling MLP + softmax +
weighted sum. The softmax is computed in one streaming pass without a
global max: |score| <= sum|ctx_w| =: c0 because tanh output is in
[-1, 1], so we subtract the host-known constant c0 (any uniform shift
cancels exactly in the softmax ratio). Normalization by 1/sum(exp) and
the final sum over the 4 chunk-partials are done on host as part of the
unshard (O(C) work per core).

General path (gamma != 0, or a pathological ctx_w norm): exact NumPy
fallback implementing the full reference math.

Device dataflow per core (bf16 data / fp32 accumulation):
  - x [256, 4096] bf16 streams as 8 [128, 1024] tiles, all DMAs issued
    up-front: channel-half 0 on the sync HWDGE queue, half 1 on the
    vector HWDGE queue (parallel ~630ns issue slots), weights on the
    scalar queue -- no serial-issue stall on one sequencer.
  - hid = tanh(fc_w @ x + fc_b) per [128, 1024] chunk (4 matmuls into a
    2-bank PSUM tile, one wide tanh).
  - score matmul uses ctx_w REPLICATED into all 128 lhsT columns, so the
    [128, 1024] PSUM result has the score row broadcast across all
    partitions at the same PE cost as an M=1 matmul (cost = streamed
    columns). exp runs wide ([128, 1024], same wall time as [1, 1024])
    and its per-partition accumulator gives the softmax denominator; the
    broadcast stage of the old design disappears entirely.
  - weighted sum: one scalar_tensor_tensor per channel-half per chunk
    ([128, 1024] multiply with free-dim accumulate into part[:, chunk]).
  - PE warm-up burst of dummy bf16 matmuls fills the dead DMA window and
    ramps the PE p-state (0.65 -> 1.2 -> 2.4 GHz after 3us busy).
  - tail: two PE transposes pack part0/part1 into a [4, 256] row set,
    ACT copies PSUM->SBUF, single 4-descriptor store DMA (+ a 16-byte
    den DMA); host sums 4 partials and divides by the denominator.
  - Kernel tail is drain-only: Tile's stock tail (all-engine barrier +
    sem clears + barrier) costs ~5us and protects nothing here; NRT
    re-initializes semaphores per execution.
"""

import numpy as np

B, C, H, W = 8, 256, 64, 64
N = H * W          # 4096
FD = C // 8        # 32
HID = 128
# Pipeline unit widths (pixels). 256-px units at both ends shorten the
# fill latency (first tanh starts sooner) and the drain chain (last
# unit's tanh->score->exp->multiply chain is half as long); 512-px units
# in the middle keep per-op overhead low.
UNITS = (256, 256, 512, 512, 512, 512, 512, 512, 256, 256)
NU = len(UNITS)
NBIG = 4           # retained for test.py compatibility
N_CORES = 8
N_WARM = 4         # PE warm-up matmuls (512 cols each)

_FAST = {}  # memoized compiled program


def _build_fast_nc():
    """Build + compile the Bass/Tile program for the gamma==0 fast path.

    Per-core I/O (one batch element per core):
      x      [256, 4096] bf16  batch slice, channels-major
      wpack  [256, 256]  bf16  cols 0:128 = fc_w^T; rows 0:128 of cols
                               128:256 = ctx_w replicated into 128 cols
      bpack  [128, 2]    f32   col 0 = fc_b, col 1 = -c0 (all rows)
      out    [128, 20]   f32   per-unit weighted-sum partials:
                               col 2u = channels 0:128 of unit u,
                               col 2u+1 = channels 128:256 (host sums)
      den    [1, 10]     f32   per-unit exp sums (host sums)

    The kernel is a pipeline over pixel units (UNITS widths):
      hid(u)  = 2 matmuls into a [128,w] PSUM tile + tanh
      rest(u) = replicated-ctxw score matmul + exp + gpsimd den-reduce +
                2 weighted-sum multiply-accumulates on DVE
    with one-unit skew (hid(u+1) emitted before rest(u)) so the PE never
    waits on the ACT round trip. All x DMAs go on the sync queue in
    need-order (c3 on scalar) -- spreading them over queues lets
    late-needed data jump the shared DMA wire ahead of early-needed
    data (measured 2.4us PE stall).
    """
    import concourse.bass as bass
    import concourse.bacc as bacc
    import concourse.tile as tile
    from concourse import mybir
    from concourse.vector_clock import ScopedClock

    f32 = mybir.dt.float32
    bf16 = mybir.dt.bfloat16
    AF = mybir.ActivationFunctionType
    ALU = mybir.AluOpType

    class _SlimTailTC(tile.TileContext):
        # Drain-only kernel tail (see module docstring).
        def _drain_and_barrier(self, tick_clock, wait_clock):
            drain_inst = self.nc.sync.drain()
            wait_clock.add_sem_waits(
                drain_inst.ins, ScopedClock({None: tick_clock.global_clock})
            )
            popped = self.nc._tile_sem_poison_stack.pop()
            assert popped is self._sem_poison

    nc = bacc.Bacc("TRN2", target_bir_lowering=False, debug=False, num_devices=1)

    x_d = nc.dram_tensor("x", [C, N], bf16, kind="ExternalInput")
    wpack_d = nc.dram_tensor("wpack", [C, 2 * HID], bf16, kind="ExternalInput")
    bpack_d = nc.dram_tensor("bpack", [HID, 2], f32, kind="ExternalInput")
    out_d = nc.dram_tensor("out", [128, 2 * NU], f32, kind="ExternalOutput")
    den_d = nc.dram_tensor("den", [1, N], bf16, kind="ExternalOutput")

    # pixel offset of each unit
    offs = [0]
    for w in UNITS:
        offs.append(offs[-1] + w)
    assert offs[-1] == N

    with _SlimTailTC(nc) as tc:
        with (
            tc.tile_pool(name="const", bufs=1) as cpool,
            tc.tile_pool(name="xc", bufs=1) as xpool,
            tc.tile_pool(name="hid", bufs=3) as hpool,
            tc.tile_pool(name="scr", bufs=3) as spool,
            tc.tile_pool(name="acc", bufs=1) as apool,
            tc.tile_pool(name="ps_h", bufs=3, space="PSUM") as ps_h,
            tc.tile_pool(name="ps_s", bufs=3, space="PSUM") as ps_s,
        ):
            # Warm-up constants on the (otherwise idle) gpsimd engine.
            ones1 = cpool.tile([1, 128], bf16)
            onesN = cpool.tile([1, 512], bf16)
            nc.gpsimd.memset(ones1, 1.0)
            nc.gpsimd.memset(onesN, 1.0)

            # Weights + the last x chunk on the scalar-engine HWDGE queue
            # (its sequencer is free until the first tanh; the ACT table
            # load overlaps the issue slots).
            wp0 = cpool.tile([128, 2 * HID], bf16)
            wp1 = cpool.tile([128, HID], bf16)
            bp = cpool.tile([HID, 2], f32)
            nc.scalar.dma_start(out=wp0, in_=wpack_d[0:128, :])
            nc.scalar.dma_start(out=wp1, in_=wpack_d[128:256, 0:HID])
            nc.scalar.dma_start(out=bp, in_=bpack_d[:, :])
            fcw0 = wp0[:, 0:HID]
            fcw1 = wp1[:, :]
            ctxw_rep = wp0[:, HID : 2 * HID]
            fcb = bp[:, 0:1]
            negc0 = bp[:, 1:2]

            # x tiles. Sync queue carries everything up to px 3072 in
            # need-order; the last 1024 px ride the scalar queue (its
            # descriptors may jump the wire, which is harmless for
            # late-needed data).
            def xpair(name, lo, hi, engine):
                t0 = xpool.tile([128, hi - lo], bf16, tag=f"{name}h0")
                t1 = xpool.tile([128, hi - lo], bf16, tag=f"{name}h1")
                engine.dma_start(out=t0, in_=x_d[0:128, lo:hi])
                engine.dma_start(out=t1, in_=x_d[128:256, lo:hi])
                return t0, t1

            t0p = xpair("t0", 0, 256, nc.sync)
            t1p = xpair("t1", 256, 512, nc.sync)
            ap = xpair("a", 512, 1536, nc.sync)
            bpx = xpair("b", 1536, 2560, nc.sync)
            cp = xpair("c", 2560, 3072, nc.sync)
            dp = xpair("d", 3072, 4096, nc.sync)

            # unit -> (xh0 view, xh1 view)
            tiles = {0: (t0p, 0), 1: (t1p, 256), 2: (ap, 512), 3: (ap, 512),
                     4: (bpx, 1536), 5: (bpx, 1536), 6: (cp, 2560),
                     7: (dp, 3072), 8: (dp, 3072), 9: (dp, 3072)}
            xh0 = [None] * NU
            xh1 = [None] * NU
            for u in range(NU):
                (th0, th1), base = tiles[u]
                s = slice(offs[u] - base, offs[u + 1] - base)
                xh0[u], xh1[u] = th0[:, s], th1[:, s]

            # PE warm-up: dummy bf16 matmuls fill the dead window while
            # the first x slabs stream in, and ramp the PE p-state.
            for _ in range(N_WARM):
                pw = ps_s.tile([128, 512], f32, tag="psc")
                nc.tensor.matmul(pw, ones1, onesN, start=True, stop=True)

            # ACT warm-up: absorb the first-op ramp (~110ns/op) off the
            # critical path, while the x DMAs are still in flight.
            wsrc = cpool.tile([128, 8], bf16)
            nc.gpsimd.memset(wsrc, 0.25)
            wdst = cpool.tile([128, 8], bf16)
            nc.scalar.activation(wdst, wsrc, AF.Tanh)
            nc.scalar.activation(wdst, wsrc, AF.Exp)

            # exp values land in a persistent arena; all 128 partitions
            # hold the same row, so row 0 ships out once at the end and
            # the host sums it for the softmax denominator (zero device
            # ops for den).
            ex_arena = apool.tile([128, N], bf16)
            part = apool.tile([128, 2 * NU], f32)

            def stage_hid(u):
                w = UNITS[u]
                ph = ps_h.tile([HID, w], f32, tag="ph")
                nc.tensor.matmul(ph, fcw0, xh0[u], start=True, stop=False)
                nc.tensor.matmul(ph, fcw1, xh1[u], start=False, stop=True)
                hid = hpool.tile([HID, w], bf16, tag="hid")
                nc.scalar.activation(hid, ph, AF.Tanh, bias=fcb)
                return hid

            def stage_rest(u, hid):
                w = UNITS[u]
                # score matmul with ctx_w replicated into all 128 lhsT
                # columns: the PSUM tile holds the score row broadcast
                # across all 128 partitions (same PE cost as M=1).
                psc = ps_s.tile([128, w], f32, tag="psc")
                nc.tensor.matmul(psc, ctxw_rep, hid, start=True, stop=True)
                ex = ex_arena[:, offs[u] : offs[u + 1]]
                nc.scalar.activation(ex, psc, AF.Exp, bias=negc0)
                # part[c, 2u+h] = sum_n x[c, n] * e[n] (mul + accum, DVE)
                s0 = spool.tile([128, w], bf16, tag="s0")
                s1 = spool.tile([128, w], bf16, tag="s1")
                nc.vector.scalar_tensor_tensor(
                    out=s0, in0=xh0[u], scalar=1.0, in1=ex,
                    op0=ALU.mult, op1=ALU.mult,
                    accum_out=part[:, 2 * u : 2 * u + 1],
                )
                nc.vector.scalar_tensor_tensor(
                    out=s1, in0=xh1[u], scalar=1.0, in1=ex,
                    op0=ALU.mult, op1=ALU.mult,
                    accum_out=part[:, 2 * u + 1 : 2 * u + 2],
                )

            prev = stage_hid(0)
            for u in range(1, NU):
                cur = stage_hid(u)
                stage_rest(u - 1, prev)
                prev = cur
            stage_rest(NU - 1, prev)

            # Tail: ship the raw partials; host does the O(C) reduction.
            # The bulk of part goes out as soon as unit NU-2 is done;
            # only the last unit's two columns ride the final DMA.
            nc.sync.dma_start(out=den_d[:, :], in_=ex_arena[0:1, :])
            nc.sync.dma_start(
                out=out_d[:, 0 : 2 * NU - 2], in_=part[:, 0 : 2 * NU - 2]
            )
            nc.sync.dma_start(
                out=out_d[:, 2 * NU - 2 : 2 * NU],
                in_=part[:, 2 * NU - 2 : 2 * NU],
            )

    nc.compile()
    return nc


def _get_fast_nc():
    if "nc" not in _FAST:
        _FAST["nc"] = _build_fast_nc()
    return _FAST["nc"]


def _make_in_maps(xf, fc_w, fc_b, ctx_w):
    import ml_dtypes

    bf16 = ml_dtypes.bfloat16
    wpack = np.zeros((C, 2 * HID), dtype=bf16)
    wpack[:, 0:HID] = fc_w.T.astype(bf16)
    wpack[0:HID, HID : 2 * HID] = np.broadcast_to(
        ctx_w.reshape(HID, 1).astype(bf16), (HID, HID)
    )
    bpack = np.zeros((HID, 2), dtype=np.float32)
    bpack[:, 0] = fc_b
    bpack[:, 1] = -float(np.abs(ctx_w).sum())
    x_bf = np.ascontiguousarray(xf).astype(bf16)
    return [
        {"x": x_bf[b], "wpack": wpack, "bpack": bpack}
        for b in range(x_bf.shape[0])
    ]


def _fast_path(xf, fc_w, fc_b, ctx_w, trace=False):
    """xf: [B, C, N] f32. Returns [B, C] f32 (and BassKernelResults if trace)."""
    from concourse.bass_utils import run_bass_kernel_spmd

    nc = _get_fast_nc()
    in_maps = _make_in_maps(xf, fc_w, fc_b, ctx_w)
    res = run_bass_kernel_spmd(nc, in_maps, list(range(N_CORES)), trace=trace)
    out = np.empty((B, C), dtype=np.float32)
    for b in range(B):
        parts = np.asarray(res.results[b]["out"], dtype=np.float32)  # [128, 2*NU]
        den = np.asarray(res.results[b]["den"]).astype(np.float64).sum()
        out[b, 0:128] = parts[:, 0::2].sum(axis=1) / den
        out[b, 128:256] = parts[:, 1::2].sum(axis=1) / den
    if trace:
        return out, res
    return out


def _general_path(x, wq, bq, wk, bk, wv, bv, gamma, fc_w, fc_b, ctx_w):
    """Exact NumPy implementation of the full reference (any gamma)."""
    x = np.asarray(x, np.float32)
    b, c, h, w = x.shape
    n = h * w
    xf = x.reshape(b, c, n)
    out = np.empty((b, c), dtype=np.float32)
    for i in range(b):
        xi = xf[i]  # [C, N]
        q = (wq @ xi).T + bq[None, :]            # [N, FD]
        k = (wk @ xi) + bk[:, None]              # [FD, N]
        logits = q @ k                           # [N, N]
        logits -= logits.max(axis=1, keepdims=True)
        e = np.exp(logits, dtype=np.float32)
        attn = e / e.sum(axis=1, keepdims=True)
        v = (wv @ xi) + bv[:, None]              # [C, N]
        sa = v @ attn.T                          # [C, N]
        h_sa = gamma.reshape(-1)[0] * sa + xi    # [C, N]
        hid = np.tanh(fc_w @ h_sa + fc_b[:, None])   # [HID, N]
        s = (ctx_w @ hid).reshape(n)             # [N]
        s = s - s.max()
        es = np.exp(s, dtype=np.float32)
        p = es / es.sum()
        out[i] = xi @ p
    return out


def kernel(**inputs):
    x = np.asarray(inputs["style_features"], np.float32)
    gamma = np.asarray(inputs["gamma"], np.float32)
    fc_w = np.asarray(inputs["fc_w"], np.float32)
    fc_b = np.asarray(inputs["fc_b"], np.float32)
    ctx_w = np.asarray(inputs["ctx_w"], np.float32)

    assert x.shape == (B, C, H, W), f"unexpected shape {x.shape}"
    c0 = float(np.abs(ctx_w).sum())

    if np.all(gamma == 0.0) and c0 <= 40.0 and np.isfinite(c0):
        # gamma == 0  =>  h_sa == x exactly; attention branch contributes 0.
        xf = x.reshape(B, C, N)
        return _fast_path(xf, fc_w, fc_b, ctx_w)

    return _general_path(
        x,
        np.asarray(inputs["wq"], np.float32),
        np.asarray(inputs["bq"], np.float32),
        np.asarray(inputs["wk"], np.float32),
        np.asarray(inputs["bk"], np.float32),
        np.asarray(inputs["wv"], np.float32),
        np.asarray(inputs["bv"], np.float32),
        gamma,
        fc_w,
        fc_b,
        ctx_w,
    )


# revision 21
# speedup vs baseline: 1.1430x; 1.0341x over previous
"""
Trainium2 (Bass/Tile) kernel for nn_ContextAwareAttentionBlock.

Problem shapes (hardcoded, from the problem spec):
    B=8, C=256, H=W=64  -> N = H*W = 4096 pixels per batch
    FD=32 (q/k feature dim), HID=128 (pooling MLP hidden dim)

Reference math:
    xf   = x.reshape(B, C, N)
    q,k,v = 1x1 convs of xf;  attn = softmax(q @ k);  sa_out = v @ attn^T
    h_sa = gamma * sa_out + x                      # gamma is a learned scalar
    hid  = tanh(fc_w @ h_sa + fc_b)                # [HID, N] per batch
    s    = softmax(ctx_w @ hid)  over N            # [N]    per batch
    out[b, c] = sum_n x[b, c, n] * s[b, n]         # [B, C]

Sharding: pure data-parallel over batch B across the 8 NeuronCores (one
batch element per core, full weights replicated) -- no cross-device
attention traffic; tiny per-core outputs gathered on host.

Fast path (gamma == 0): the module initializes gamma = zeros(1), so
h_sa == x *exactly* and the whole self-attention branch is multiplied by
zero. The device kernel then only needs the pooling MLP + softmax +
weighted sum. The softmax is computed in one streaming pass without a
global max: |score| <= sum|ctx_w| =: c0 because tanh output is in
[-1, 1], so we subtract the host-known constant c0 (any uniform shift
cancels exactly in the softmax ratio). Normalization by 1/sum(exp) and
the final sum over the 4 chunk-partials are done on host as part of the
unshard (O(C) work per core).

General path (gamma != 0, or a pathological ctx_w norm): exact NumPy
fallback implementing the full reference math.

Device dataflow per core (bf16 data / fp32 accumulation):
  - x [256, 4096] bf16 streams as 8 [128, 1024] tiles, all DMAs issued
    up-front: channel-half 0 on the sync HWDGE queue, half 1 on the
    vector HWDGE queue (parallel ~630ns issue slots), weights on the
    scalar queue -- no serial-issue stall on one sequencer.
  - hid = tanh(fc_w @ x + fc_b) per [128, 1024] chunk (4 matmuls into a
    2-bank PSUM tile, one wide tanh).
  - score matmul uses ctx_w REPLICATED into all 128 lhsT columns, so the
    [128, 1024] PSUM result has the score row broadcast across all
    partitions at the same PE cost as an M=1 matmul (cost = streamed
    columns). exp runs wide ([128, 1024], same wall time as [1, 1024])
    and its per-partition accumulator gives the softmax denominator; the
    broadcast stage of the old design disappears entirely.
  - weighted sum: one scalar_tensor_tensor per channel-half per chunk
    ([128, 1024] multiply with free-dim accumulate into part[:, chunk]).
  - PE warm-up burst of dummy bf16 matmuls fills the dead DMA window and
    ramps the PE p-state (0.65 -> 1.2 -> 2.4 GHz after 3us busy).
  - tail: two PE transposes pack part0/part1 into a [4, 256] row set,
    ACT copies PSUM->SBUF, single 4-descriptor store DMA (+ a 16-byte
    den DMA); host sums 4 partials and divides by the denominator.
  - Kernel tail is drain-only: Tile's stock tail (all-engine barrier +
    sem clears + barrier) costs ~5us and protects nothing here; NRT
    re-initializes semaphores per execution.
"""

import numpy as np

B, C, H, W = 8, 256, 64, 64
N = H * W          # 4096
FD = C // 8        # 32
HID = 128
# Pipeline unit widths (pixels). 256-px units at both ends shorten the
# fill latency (first tanh starts sooner) and the drain chain (last
# unit's tanh->score->exp->multiply chain is half as long); 512-px units
# in the middle keep per-op overhead low.
UNITS = (256, 256, 512, 512, 512, 512, 512, 512, 256, 256)
NU = len(UNITS)
NBIG = 4           # retained for test.py compatibility
N_CORES = 8
N_WARM = 4         # PE warm-up matmuls (512 cols each)

_FAST = {}  # memoized compiled program


def _build_fast_nc():
    """Build + compile the Bass/Tile program for the gamma==0 fast path.

    Per-core I/O (one batch element per core):
      x      [256, 4096] bf16  batch slice, channels-major
      wpack  [256, 256]  bf16  cols 0:128 = fc_w^T; rows 0:128 of cols
                               128:256 = ctx_w replicated into 128 cols
      bpack  [128, 2]    f32   col 0 = fc_b, col 1 = -c0 (all rows)
      out    [128, 20]   f32   per-unit weighted-sum partials:
                               col 2u = channels 0:128 of unit u,
                               col 2u+1 = channels 128:256 (host sums)
      den    [1, 10]     f32   per-unit exp sums (host sums)

    The kernel is a pipeline over pixel units (UNITS widths):
      hid(u)  = 2 matmuls into a [128,w] PSUM tile + tanh
      rest(u) = replicated-ctxw score matmul + exp + gpsimd den-reduce +
                2 weighted-sum multiply-accumulates on DVE
    with one-unit skew (hid(u+1) emitted before rest(u)) so the PE never
    waits on the ACT round trip. All x DMAs go on the sync queue in
    need-order (c3 on scalar) -- spreading them over queues lets
    late-needed data jump the shared DMA wire ahead of early-needed
    data (measured 2.4us PE stall).
    """
    import concourse.bass as bass
    import concourse.bacc as bacc
    import concourse.tile as tile
    from concourse import mybir
    from concourse.vector_clock import ScopedClock

    f32 = mybir.dt.float32
    bf16 = mybir.dt.bfloat16
    AF = mybir.ActivationFunctionType
    ALU = mybir.AluOpType

    class _SlimTailTC(tile.TileContext):
        # Drain-only kernel tail (see module docstring).
        def _drain_and_barrier(self, tick_clock, wait_clock):
            drain_inst = self.nc.sync.drain()
            wait_clock.add_sem_waits(
                drain_inst.ins, ScopedClock({None: tick_clock.global_clock})
            )
            popped = self.nc._tile_sem_poison_stack.pop()
            assert popped is self._sem_poison

    nc = bacc.Bacc("TRN2", target_bir_lowering=False, debug=False, num_devices=1)

    x_d = nc.dram_tensor("x", [C, N], bf16, kind="ExternalInput")
    wpack_d = nc.dram_tensor("wpack", [C, 2 * HID], bf16, kind="ExternalInput")
    bpack_d = nc.dram_tensor("bpack", [HID, 2], f32, kind="ExternalInput")
    out_d = nc.dram_tensor("out", [128, 2 * NU], f32, kind="ExternalOutput")
    den_d = nc.dram_tensor("den", [1, N], bf16, kind="ExternalOutput")

    # pixel offset of each unit
    offs = [0]
    for w in UNITS:
        offs.append(offs[-1] + w)
    assert offs[-1] == N

    with _SlimTailTC(nc) as tc:
        with (
            tc.tile_pool(name="const", bufs=1) as cpool,
            tc.tile_pool(name="xc", bufs=1) as xpool,
            tc.tile_pool(name="hid", bufs=3) as hpool,
            tc.tile_pool(name="scr", bufs=3) as spool,
            tc.tile_pool(name="acc", bufs=1) as apool,
            tc.tile_pool(name="ps_h", bufs=3, space="PSUM") as ps_h,
            tc.tile_pool(name="ps_s", bufs=3, space="PSUM") as ps_s,
        ):
            # Warm-up constants on the (otherwise idle) gpsimd engine.
            ones1 = cpool.tile([1, 128], bf16)
            onesN = cpool.tile([1, 512], bf16)
            nc.gpsimd.memset(ones1, 1.0)
            nc.gpsimd.memset(onesN, 1.0)

            # Weights + the last x chunk on the scalar-engine HWDGE queue
            # (its sequencer is free until the first tanh; the ACT table
            # load overlaps the issue slots).
            wp0 = cpool.tile([128, 2 * HID], bf16)
            wp1 = cpool.tile([128, HID], bf16)
            bp = cpool.tile([HID, 2], f32)
            nc.scalar.dma_start(out=wp0, in_=wpack_d[0:128, :])
            nc.scalar.dma_start(out=wp1, in_=wpack_d[128:256, 0:HID])
            nc.scalar.dma_start(out=bp, in_=bpack_d[:, :])
            fcw0 = wp0[:, 0:HID]
            fcw1 = wp1[:, :]
            ctxw_rep = wp0[:, HID : 2 * HID]
            fcb = bp[:, 0:1]
            negc0 = bp[:, 1:2]

            # x tiles. Sync queue carries everything up to px 3072 in
            # need-order; the last 1024 px ride the scalar queue (its
            # descriptors may jump the wire, which is harmless for
            # late-needed data).
            def xpair(name, lo, hi, engine):
                t0 = xpool.tile([128, hi - lo], bf16, tag=f"{name}h0")
                t1 = xpool.tile([128, hi - lo], bf16, tag=f"{name}h1")
                engine.dma_start(out=t0, in_=x_d[0:128, lo:hi])
                engine.dma_start(out=t1, in_=x_d[128:256, lo:hi])
                return t0, t1

            # Every [128, w] DMA costs 128 descriptors (~85ns/desc queue
            # latency) regardless of w, so narrow tiles waste wire time:
            # keep tiles >= 512 px and subdivide into units via views.
            tp = xpair("t", 0, 512, nc.sync)
            ap = xpair("a", 512, 1536, nc.sync)
            bpx = xpair("b", 1536, 2560, nc.sync)
            cp = xpair("c", 2560, 3584, nc.sync)
            dp = xpair("d", 3584, 4096, nc.sync)

            # unit -> (xh0 view, xh1 view)
            tiles = {0: (tp, 0), 1: (tp, 0), 2: (ap, 512), 3: (ap, 512),
                     4: (bpx, 1536), 5: (bpx, 1536), 6: (cp, 2560),
                     7: (cp, 2560), 8: (dp, 3584), 9: (dp, 3584)}
            xh0 = [None] * NU
            xh1 = [None] * NU
            for u in range(NU):
                (th0, th1), base = tiles[u]
                s = slice(offs[u] - base, offs[u + 1] - base)
                xh0[u], xh1[u] = th0[:, s], th1[:, s]

            # PE warm-up: dummy bf16 matmuls fill the dead window while
            # the first x slabs stream in, and ramp the PE p-state.
            for _ in range(N_WARM):
                pw = ps_s.tile([128, 512], f32, tag="psc")
                nc.tensor.matmul(pw, ones1, onesN, start=True, stop=True)

            # ACT warm-up: absorb the first-op ramp (~110ns/op) off the
            # critical path, while the x DMAs are still in flight.
            wsrc = cpool.tile([128, 8], bf16)
            nc.gpsimd.memset(wsrc, 0.25)
            wdst = cpool.tile([128, 8], bf16)
            nc.scalar.activation(wdst, wsrc, AF.Tanh)
            nc.scalar.activation(wdst, wsrc, AF.Exp)

            # exp values land in a persistent arena; all 128 partitions
            # hold the same row, so row 0 ships out once at the end and
            # the host sums it for the softmax denominator (zero device
            # ops for den).
            ex_arena = apool.tile([128, N], bf16)
            part = apool.tile([128, 2 * NU], f32)

            def stage_hid(u):
                w = UNITS[u]
                ph = ps_h.tile([HID, w], f32, tag="ph")
                nc.tensor.matmul(ph, fcw0, xh0[u], start=True, stop=False)
                nc.tensor.matmul(ph, fcw1, xh1[u], start=False, stop=True)
                hid = hpool.tile([HID, w], bf16, tag="hid")
                nc.scalar.activation(hid, ph, AF.Tanh, bias=fcb)
                return hid

            def stage_rest(u, hid):
                w = UNITS[u]
                # score matmul with ctx_w replicated into all 128 lhsT
                # columns: the PSUM tile holds the score row broadcast
                # across all 128 partitions (same PE cost as M=1).
                psc = ps_s.tile([128, w], f32, tag="psc")
                nc.tensor.matmul(psc, ctxw_rep, hid, start=True, stop=True)
                ex = ex_arena[:, offs[u] : offs[u + 1]]
                nc.scalar.activation(ex, psc, AF.Exp, bias=negc0)
                # part[c, 2u+h] = sum_n x[c, n] * e[n] (mul + accum, DVE)
                s0 = spool.tile([128, w], bf16, tag="s0")
                s1 = spool.tile([128, w], bf16, tag="s1")
                nc.vector.scalar_tensor_tensor(
                    out=s0, in0=xh0[u], scalar=1.0, in1=ex,
                    op0=ALU.mult, op1=ALU.mult,
                    accum_out=part[:, 2 * u : 2 * u + 1],
                )
                nc.vector.scalar_tensor_tensor(
                    out=s1, in0=xh1[u], scalar=1.0, in1=ex,
                    op0=ALU.mult, op1=ALU.mult,
                    accum_out=part[:, 2 * u + 1 : 2 * u + 2],
                )

            prev = stage_hid(0)
            for u in range(1, NU):
                cur = stage_hid(u)
                stage_rest(u - 1, prev)
                prev = cur
            stage_rest(NU - 1, prev)

            # Tail: ship the raw partials; host does the O(C) reduction.
            # The bulk of part goes out as soon as unit NU-2 is done;
            # only the last unit's two columns ride the final DMA.
            nc.sync.dma_start(out=den_d[:, :], in_=ex_arena[0:1, :])
            nc.sync.dma_start(
                out=out_d[:, 0 : 2 * NU - 2], in_=part[:, 0 : 2 * NU - 2]
            )
            nc.sync.dma_start(
                out=out_d[:, 2 * NU - 2 : 2 * NU],
                in_=part[:, 2 * NU - 2 : 2 * NU],
            )

    nc.compile()
    return nc


def _get_fast_nc():
    if "nc" not in _FAST:
        _FAST["nc"] = _build_fast_nc()
    return _FAST["nc"]


def _make_in_maps(xf, fc_w, fc_b, ctx_w):
    import ml_dtypes

    bf16 = ml_dtypes.bfloat16
    wpack = np.zeros((C, 2 * HID), dtype=bf16)
    wpack[:, 0:HID] = fc_w.T.astype(bf16)
    wpack[0:HID, HID : 2 * HID] = np.broadcast_to(
        ctx_w.reshape(HID, 1).astype(bf16), (HID, HID)
    )
    bpack = np.zeros((HID, 2), dtype=np.float32)
    bpack[:, 0] = fc_b
    bpack[:, 1] = -float(np.abs(ctx_w).sum())
    x_bf = np.ascontiguousarray(xf).astype(bf16)
    return [
        {"x": x_bf[b], "wpack": wpack, "bpack": bpack}
        for b in range(x_bf.shape[0])
    ]


def _fast_path(xf, fc_w, fc_b, ctx_w, trace=False):
    """xf: [B, C, N] f32. Returns [B, C] f32 (and BassKernelResults if trace)."""
    from concourse.bass_utils import run_bass_kernel_spmd

    nc = _get_fast_nc()
    in_maps = _make_in_maps(xf, fc_w, fc_b, ctx_w)
    res = run_bass_kernel_spmd(nc, in_maps, list(range(N_CORES)), trace=trace)
    out = np.empty((B, C), dtype=np.float32)
    for b in range(B):
        parts = np.asarray(res.results[b]["out"], dtype=np.float32)  # [128, 2*NU]
        den = np.asarray(res.results[b]["den"]).astype(np.float64).sum()
        out[b, 0:128] = parts[:, 0::2].sum(axis=1) / den
        out[b, 128:256] = parts[:, 1::2].sum(axis=1) / den
    if trace:
        return out, res
    return out


def _general_path(x, wq, bq, wk, bk, wv, bv, gamma, fc_w, fc_b, ctx_w):
    """Exact NumPy implementation of the full reference (any gamma)."""
    x = np.asarray(x, np.float32)
    b, c, h, w = x.shape
    n = h * w
    xf = x.reshape(b, c, n)
    out = np.empty((b, c), dtype=np.float32)
    for i in range(b):
        xi = xf[i]  # [C, N]
        q = (wq @ xi).T + bq[None, :]            # [N, FD]
        k = (wk @ xi) + bk[:, None]              # [FD, N]
        logits = q @ k                           # [N, N]
        logits -= logits.max(axis=1, keepdims=True)
        e = np.exp(logits, dtype=np.float32)
        attn = e / e.sum(axis=1, keepdims=True)
        v = (wv @ xi) + bv[:, None]              # [C, N]
        sa = v @ attn.T                          # [C, N]
        h_sa = gamma.reshape(-1)[0] * sa + xi    # [C, N]
        hid = np.tanh(fc_w @ h_sa + fc_b[:, None])   # [HID, N]
        s = (ctx_w @ hid).reshape(n)             # [N]
        s = s - s.max()
        es = np.exp(s, dtype=np.float32)
        p = es / es.sum()
        out[i] = xi @ p
    return out


def kernel(**inputs):
    x = np.asarray(inputs["style_features"], np.float32)
    gamma = np.asarray(inputs["gamma"], np.float32)
    fc_w = np.asarray(inputs["fc_w"], np.float32)
    fc_b = np.asarray(inputs["fc_b"], np.float32)
    ctx_w = np.asarray(inputs["ctx_w"], np.float32)

    assert x.shape == (B, C, H, W), f"unexpected shape {x.shape}"
    c0 = float(np.abs(ctx_w).sum())

    if np.all(gamma == 0.0) and c0 <= 40.0 and np.isfinite(c0):
        # gamma == 0  =>  h_sa == x exactly; attention branch contributes 0.
        xf = x.reshape(B, C, N)
        return _fast_path(xf, fc_w, fc_b, ctx_w)

    return _general_path(
        x,
        np.asarray(inputs["wq"], np.float32),
        np.asarray(inputs["bq"], np.float32),
        np.asarray(inputs["wk"], np.float32),
        np.asarray(inputs["bk"], np.float32),
        np.asarray(inputs["wv"], np.float32),
        np.asarray(inputs["bv"], np.float32),
        gamma,
        fc_w,
        fc_b,
        ctx_w,
    )
